# revision 14
# baseline (speedup 1.0000x reference)
"""Trainium2 Bass kernel for nn_Log_GraphConv4d (log-shift-max + 1x1 conv + BN + GeLU).

Math refactor (validated in numpy):
  x_j = x - m,  m = min(x, 20 circular rolls of x: +-{1,3,7,15,31} on h and w)
  y = gelu(a * (Wc @ [x; m]) + b)   with host-folded weights/BN params.

The 21-shift min is a 12-op DVE tensor_tensor MIN DAG (exhaustive-search
optimal: 5 ops per axis arm using mod-56 wraparound +-31 == -+25, plus two
merge ops).  W-axis implementation shifts are all even (DVE 2x-mode 4-byte
alignment); odd positions enter through X1 = roll(x, 1 in w), loaded from
HBM as a flat-shifted copy + last-column fixup.  The two merge ops are
emitted as 7 row-slab pieces so the tensor engine's n-tile waves can chase
the DVE down the image.

Matmul: n-outer waves, 3 PSUM banks per wave (o0..o2), k-inner with x
chunks first; gelu+affine fused on ScalarE; bf16 output, upcast on host.

Sharding: data-parallel over batch, 2 samples per core on 8 cores.
"""

import numpy as np
import ml_dtypes

import concourse.bass as bass
import concourse.mybir as mybir
from concourse import bacc
from concourse.tile import TileContext
from concourse.bass_utils import run_bass_kernel_spmd

N_CORES = 8
B, C, H, W = 16, 384, 56, 56
HW = H * W                 # 3136
B_LOC = B // N_CORES       # 2 samples per core
CC = C // 128              # 3 input-channel chunks
OC = 384 // 128            # 3 output-channel chunks
NT = 7                     # n-tile waves; each = 8 rows of 56 = 448 cols
NF = HW // NT              # 448
HS = H // NT               # 8 rows per slab

BF16 = mybir.dt.bfloat16
F32 = mybir.dt.float32
MIN = mybir.AluOpType.min
GELU = mybir.ActivationFunctionType.Gelu

LAST_RESULTS = None

# --- min-DAG (search-optimal; see dag_search56.py) ------------------------
# op = (out, srcA, shiftA, srcB, shiftB, axis)
# out[w] = min(srcA[(w+shiftA) % 56], srcB[(w+shiftB) % 56]) along axis.
# Leaves: 'X', 'X1' (X1[w] = x[w+1] along w).
CHAIN_OPS = [
    # H arm {+-1,+-3,+-7,+-15,+-25==-+31}: 5 ops
    ("h0", "X", 0, "X", 2, "h"),
    ("h1", "X", 0, "X", 10, "h"),
    ("h2", "h1", 0, "h1", 18, "h"),
    ("h3", "h2", 6, "h2", 28, "h"),
    ("h4", "h3", 25, "h0", 55, "h"),
    # W arm: 5 ops, all-even shifts via X1
    ("w0", "X1", 0, "X1", 2, "w"),
    ("w1", "X1", 0, "X1", 10, "w"),
    ("w2", "w1", 0, "w1", 18, "w"),
    ("w3", "w2", 6, "w2", 28, "w"),
    ("w4", "w3", 24, "w0", 54, "w"),
]
# merge ops, emitted as 7 row-slab pieces each:
#   mm = min(w4, h4); m = min(mm, X)


def _dag_check():
    """numpy brute-force check of chain tables + merges (2D, circular)."""
    rng = np.random.default_rng(0)
    x = rng.standard_normal((H, W)).astype(np.float32)
    vals = {"X": x, "X1": np.roll(x, -1, axis=1)}
    for out, a, sa, b, sb, axis in CHAIN_OPS:
        ax = 0 if axis == "h" else 1
        vals[out] = np.minimum(np.roll(vals[a], -sa, axis=ax),
                               np.roll(vals[b], -sb, axis=ax))
    m = np.minimum(np.minimum(vals["w4"], vals["h4"]), x)
    expect = x.copy()
    for s in (1, 3, 7, 15, 31):
        for ax in (0, 1):
            for sgn in (s, -s):
                expect = np.minimum(expect, np.roll(x, -sgn, axis=ax))
    assert np.allclose(m, expect), "DAG table check failed"


def _emit_min(eng, out, a, sa, b, sb, axis, L=56):
    """out = min(roll(a, sa, axis), roll(b, sb, axis)) on [128, cc, H, W]
    tiles; roll(t, s)[i] = t[(i+s) mod L]. Circular wrap via region split."""
    sa %= L
    sb %= L
    cuts = sorted({0, (L - sa) % L, (L - sb) % L})
    for idx, p in enumerate(cuts):
        q = cuts[idx + 1] if idx + 1 < len(cuts) else L
        n = q - p
        alo = (p + sa) % L
        blo = (p + sb) % L
        if axis == "w":
            eng.tensor_tensor(
                out=out[:, :, :, p:q],
                in0=a[:, :, :, alo:alo + n],
                in1=b[:, :, :, blo:blo + n],
                op=MIN,
            )
        else:
            of = out.rearrange("p c h w -> p c (h w)")
            af = a.rearrange("p c h w -> p c (h w)")
            bf = b.rearrange("p c h w -> p c (h w)")
            eng.tensor_tensor(
                out=of[:, :, p * W:q * W],
                in0=af[:, :, alo * W:(alo + n) * W],
                in1=bf[:, :, blo * W:(blo + n) * W],
                op=MIN,
            )


def _build(b_loc=B_LOC, cc=CC, oc=OC, act=GELU):
    nc = bacc.Bacc(None, target_bir_lowering=False)
    kc = 2 * cc

    xd = nc.dram_tensor("x_in", [b_loc, 128, cc, HW], BF16, kind="ExternalInput")
    x1d = nc.dram_tensor("x1_in", [b_loc, 128, cc, HW], BF16, kind="ExternalInput")
    wd = nc.dram_tensor("w_in", [kc, 128, oc * 128], BF16, kind="ExternalInput")
    idd = nc.dram_tensor("i_in", [1, 128, 128], BF16, kind="ExternalInput")
    pd = nc.dram_tensor("p_in", [oc, 128, 2], F32, kind="ExternalInput")
    yd = nc.dram_tensor("y_out", [b_loc, oc, 128, HW], BF16, kind="ExternalOutput")

    # liveness-based slot assignment for chain temporaries
    last_use = {}
    for i, (out, a, sa, b_, sb, axis) in enumerate(CHAIN_OPS):
        last_use[a] = i
        last_use[b_] = i
    last_use["h4"] = last_use["w4"] = len(CHAIN_OPS)  # used by merge pieces
    slot_of = {}
    free = []
    nslots = 0
    for i, (out, a, sa, b_, sb, axis) in enumerate(CHAIN_OPS):
        if free:
            slot_of[out] = free.pop()
        else:
            slot_of[out] = nslots
            nslots += 1
        for src in {a, b_}:
            if (src in slot_of and last_use.get(src) == i
                    and slot_of[src] not in free):
                free.append(slot_of[src])

    with TileContext(nc) as tc:
        with tc.tile_pool(name="sb", bufs=1) as pool, \
             tc.tile_pool(name="ps", bufs=8, space="PSUM") as psum:

            # --- hoisted input loads for both samples ---
            # XP = a small duplicate of chunk 0, loaded first, so the DVE
            # starts on chunk-0 pieces of h0/h1 ~3us before full X lands.
            XP = pool.tile([128, 1, H, W], BF16, tag="xp", bufs=1, name="XP")
            nc.sync.dma_start(out=XP.rearrange("p c h w -> p c (h w)"),
                              in_=xd[0][:, 0:1, :])
            Xs, X1s = [], []
            for b in range(b_loc):
                X = pool.tile([128, cc, H, W], BF16, tag="X", bufs=2, name="X")
                Xf_ = X.rearrange("p c h w -> p c (h w)")
                if b == 0:
                    nc.sync.dma_start(out=Xf_[:, 1:2, :], in_=xd[b][:, 1:2, :])
                    nc.scalar.dma_start(out=Xf_[:, 2:3, :], in_=xd[b][:, 2:3, :])
                    nc.scalar.dma_start(out=Xf_[:, 0:1, :], in_=xd[b][:, 0:1, :])
                else:
                    nc.sync.dma_start(out=Xf_, in_=xd[b])
                X1 = pool.tile([128, cc, H, W], BF16, tag="X1", bufs=1, name="X1")
                X1f = X1.rearrange("p c h w -> p c (h w)")
                # X1 = roll(x, -1 in w) is prepared on the host as its own
                # aligned input: the 2-byte-shifted view of x_in DMA'd at
                # ~half bandwidth (6270B misaligned packets) and needed a
                # col-55 fixup op; the aligned load needs neither.
                nc.sync.dma_start(out=X1f, in_=x1d[b])
                Xs.append(X)
                X1s.append(X1)

            # --- weights + params on the scalar HWDGE queue ---
            w_sb = pool.tile([128, kc, oc * 128], BF16, tag="w", bufs=1, name="w_sb")
            for k in range(kc):
                nc.scalar.dma_start(out=w_sb[:, k, :], in_=wd[k])
            prm = pool.tile([128, oc, 2], F32, tag="prm", bufs=1, name="prm")
            nc.scalar.dma_start(out=prm, in_=pd.rearrange("o p t -> p o t"))
            id_sb = pool.tile([128, 128], BF16, tag="id", bufs=1, name="id_sb")
            nc.scalar.dma_start(out=id_sb, in_=idd[0])

            for b in range(b_loc):
                X, X1 = Xs[b], X1s[b]
                Xf = X.rearrange("p c h w -> p c (h w)")

                # --- chain ops ---
                tiles = {"X": X, "X1": X1}
                for (out, a, sa, b_, sb, axis) in CHAIN_OPS:
                    t = pool.tile([128, cc, H, W], BF16,
                                  tag=f"s{slot_of[out]}", bufs=1,
                                  name=f"s{slot_of[out]}")
                    if b == 0 and out in ("h0", "h1"):
                        # chunk-0 piece from the early XP copy; rest from X
                        _emit_min(nc.vector, t[:, 0:1], XP, sa, XP, sb, axis)
                        _emit_min(nc.vector, t[:, 1:3], X[:, 1:3], sa,
                                  X[:, 1:3], sb, axis)
                    else:
                        _emit_min(nc.vector, t, tiles[a], sa, tiles[b_], sb,
                                  axis)
                    tiles[out] = t

                # --- merge ops as 7 row-slab pieces ---
                w4t, h4t = tiles["w4"], tiles["h4"]
                mp_slabs = []
                for n in range(NT):
                    r0, r1 = n * HS, (n + 1) * HS
                    mmt = pool.tile([128, cc, HS, W], BF16, tag="mmt",
                                    bufs=2, name="mmt")
                    nc.vector.tensor_tensor(
                        out=mmt, in0=w4t[:, :, r0:r1], in1=h4t[:, :, r0:r1],
                        op=MIN)
                    mpt = pool.tile([128, cc, HS, W], BF16, tag=f"mp{n}", bufs=1,
                                    name=f"mp{n}")
                    nc.vector.tensor_tensor(
                        out=mpt, in0=mmt, in1=X[:, :, r0:r1], op=MIN)
                    mp_slabs.append(mpt)

                # --- matmul, two phases ---
                # x-phase: all 21 (n, o) x-part partial sums run during the
                # DVE chain (PE is otherwise idle there); each PSUM tile is
                # spilled to SBUF as bf16 by an ACT copy.  m-phase: per wave,
                # 3 m-chunk matmuls + one identity-weight matmul that
                # accumulates the spilled x-part back into PSUM, so the
                # post-chain tail is 4 MMs/wave instead of 18.
                y_sbs = [pool.tile([128, HW], BF16, tag="y", bufs=3,
                                   name=f"y{o}") for o in range(oc)]
                yx = pool.tile([128, oc, HW], BF16, tag="yx", bufs=1,
                               name="yx")
                for n in range(NT):
                    for o in range(oc):
                        pst = psum.tile([128, NF], F32, tag="ps",
                                        name=f"x{b}_{n}_{o}")
                        for k in range(cc):
                            nc.tensor.matmul(
                                pst,
                                lhsT=w_sb[:, k, o * 128:(o + 1) * 128],
                                rhs=Xf[:, k, n * NF:(n + 1) * NF],
                                start=(k == 0),
                                stop=(k == cc - 1),
                            )
                        nc.scalar.copy(
                            out=yx[:, o, n * NF:(n + 1) * NF], in_=pst)

                for n in range(NT):
                    mslab = mp_slabs[n].rearrange("p c h w -> p c (h w)")
                    sl = slice(n * NF, (n + 1) * NF)
                    for o in range(oc):
                        pst = psum.tile([128, NF], F32, tag="ps",
                                        name=f"u{b}_{n}_{o}")
                        # identity-accumulate first: it depends only on yx
                        # (not the merge slabs), so early waves' I-matmuls
                        # pre-run while the DVE is still producing slabs.
                        nc.tensor.matmul(
                            pst, lhsT=id_sb, rhs=yx[:, o, sl],
                            start=True, stop=False,
                        )
                        for k in range(cc, kc):
                            nc.tensor.matmul(
                                pst,
                                lhsT=w_sb[:, k, o * 128:(o + 1) * 128],
                                rhs=mslab[:, k - cc],
                                start=False,
                                stop=(k == kc - 1),
                            )
                        nc.scalar.activation(
                            out=y_sbs[o][:, sl],
                            in_=pst,
                            func=act,
                            bias=prm[:, o, 1:2],
                            scale=prm[:, o, 0:1],
                        )
                        nc.sync.dma_start(out=yd[b, o][:, sl],
                                          in_=y_sbs[o][:, sl])
    nc.finalize()
    return nc


_CACHE = {}


def _get_program():
    if "nc" not in _CACHE:
        _dag_check()
        _CACHE["nc"] = _build()
    return _CACHE["nc"]


def kernel(x, conv_w, conv_b, bn_scale, bn_bias, bn_mean, bn_var,
           _trace=False, _tmpdir=None):
    global LAST_RESULTS
    x = np.asarray(x, dtype=np.float32)
    conv_w = np.asarray(conv_w, dtype=np.float32)
    conv_b = np.asarray(conv_b, dtype=np.float32)
    bn_scale = np.asarray(bn_scale, dtype=np.float32)
    bn_bias = np.asarray(bn_bias, dtype=np.float32)
    bn_mean = np.asarray(bn_mean, dtype=np.float32)
    bn_var = np.asarray(bn_var, dtype=np.float32)

    # host-side weight/param folding
    Wm = conv_w[:, :, 0, 0]                      # [384, 768]
    W1, W2 = Wm[:, :C], Wm[:, C:]
    wT = np.concatenate([(W1 + W2).T, (-W2).T], axis=0)   # [768, 384]
    wd_arr = np.ascontiguousarray(
        wT.reshape(2 * CC, 128, OC * 128).astype(ml_dtypes.bfloat16)
    )
    inv = 1.0 / np.sqrt(bn_var + 1e-5)
    a = (inv * bn_scale).astype(np.float32)
    b_aff = ((conv_b - bn_mean) * a + bn_bias).astype(np.float32)
    prm_arr = np.ascontiguousarray(
        np.stack([a.reshape(OC, 128), b_aff.reshape(OC, 128)], axis=-1)
    )
    id_arr = np.ascontiguousarray(
        np.eye(128, dtype=np.float32).reshape(1, 128, 128)
        .astype(ml_dtypes.bfloat16)
    )

    # [B, CC, 128, HW] -> [B, 128, CC, HW] so the device DMA is contiguous
    xs = np.ascontiguousarray(
        x.reshape(B, CC, 128, HW).transpose(0, 2, 1, 3)
        .astype(ml_dtypes.bfloat16)
    )
    # x1 = roll(x, -1 along w), wrap included, as an aligned input tensor
    x1s = np.ascontiguousarray(
        np.roll(xs.reshape(B, 128, CC, H, W), -1, axis=4).reshape(B, 128, CC, HW)
    )
    in_maps = []
    for core in range(N_CORES):
        sl = slice(core * B_LOC, (core + 1) * B_LOC)
        in_maps.append({"x_in": np.ascontiguousarray(xs[sl]),
                        "x1_in": np.ascontiguousarray(x1s[sl]),
                        "w_in": wd_arr, "p_in": prm_arr, "i_in": id_arr})

    nc = _get_program()
    res = run_bass_kernel_spmd(
        nc, in_maps, core_ids=list(range(N_CORES)), trace=_trace, tmpdir=_tmpdir
    )
    LAST_RESULTS = res
    y = np.concatenate([r["y_out"] for r in res.results], axis=0)
    return y.astype(np.float32).reshape(B, C, H, W)



# revision 15
# speedup vs baseline: 1.0119x; 1.0119x over previous
"""Trainium2 Bass kernel for nn_Log_GraphConv4d (log-shift-max + 1x1 conv + BN + GeLU).

Math refactor (validated in numpy):
  x_j = x - m,  m = min(x, 20 circular rolls of x: +-{1,3,7,15,31} on h and w)
  y = gelu(a * (Wc @ [x; m]) + b)   with host-folded weights/BN params.

The 21-shift min is a 12-op DVE tensor_tensor MIN DAG (exhaustive-search
optimal: 5 ops per axis arm using mod-56 wraparound +-31 == -+25, plus two
merge ops).  W-axis implementation shifts are all even (DVE 2x-mode 4-byte
alignment); odd positions enter through X1 = roll(x, 1 in w), loaded from
HBM as a flat-shifted copy + last-column fixup.  The two merge ops are
emitted as 7 row-slab pieces so the tensor engine's n-tile waves can chase
the DVE down the image.

Matmul: n-outer waves, 3 PSUM banks per wave (o0..o2), k-inner with x
chunks first; gelu+affine fused on ScalarE; bf16 output, upcast on host.

Sharding: data-parallel over batch, 2 samples per core on 8 cores.
"""

import numpy as np
import ml_dtypes

import concourse.bass as bass
import concourse.mybir as mybir
from concourse import bacc
from concourse.tile import TileContext
from concourse.bass_utils import run_bass_kernel_spmd

N_CORES = 8
B, C, H, W = 16, 384, 56, 56
HW = H * W                 # 3136
B_LOC = B // N_CORES       # 2 samples per core
CC = C // 128              # 3 input-channel chunks
OC = 384 // 128            # 3 output-channel chunks
NT = 7                     # n-tile waves; each = 8 rows of 56 = 448 cols
NF = HW // NT              # 448
HS = H // NT               # 8 rows per slab

BF16 = mybir.dt.bfloat16
F32 = mybir.dt.float32
MIN = mybir.AluOpType.min
GELU = mybir.ActivationFunctionType.Gelu

LAST_RESULTS = None

# --- min-DAG (search-optimal; see dag_search56.py) ------------------------
# op = (out, srcA, shiftA, srcB, shiftB, axis)
# out[w] = min(srcA[(w+shiftA) % 56], srcB[(w+shiftB) % 56]) along axis.
# Leaves: 'X', 'X1' (X1[w] = x[w+1] along w).
CHAIN_OPS = [
    # H arm {+-1,+-3,+-7,+-15,+-25==-+31}: 5 ops
    ("h0", "X", 0, "X", 2, "h"),
    ("h1", "X", 0, "X", 10, "h"),
    ("h2", "h1", 0, "h1", 18, "h"),
    ("h3", "h2", 6, "h2", 28, "h"),
    ("h4", "h3", 25, "h0", 55, "h"),
    # W arm: 5 ops, all-even shifts via X1
    ("w0", "X1", 0, "X1", 2, "w"),
    ("w1", "X1", 0, "X1", 10, "w"),
    ("w2", "w1", 0, "w1", 18, "w"),
    ("w3", "w2", 6, "w2", 28, "w"),
    ("w4", "w3", 24, "w0", 54, "w"),
]
# merge ops, emitted as 7 row-slab pieces each:
#   mm = min(w4, h4); m = min(mm, X)


def _dag_check():
    """numpy brute-force check of chain tables + merges (2D, circular)."""
    rng = np.random.default_rng(0)
    x = rng.standard_normal((H, W)).astype(np.float32)
    vals = {"X": x, "X1": np.roll(x, -1, axis=1)}
    for out, a, sa, b, sb, axis in CHAIN_OPS:
        ax = 0 if axis == "h" else 1
        vals[out] = np.minimum(np.roll(vals[a], -sa, axis=ax),
                               np.roll(vals[b], -sb, axis=ax))
    m = np.minimum(np.minimum(vals["w4"], vals["h4"]), x)
    expect = x.copy()
    for s in (1, 3, 7, 15, 31):
        for ax in (0, 1):
            for sgn in (s, -s):
                expect = np.minimum(expect, np.roll(x, -sgn, axis=ax))
    assert np.allclose(m, expect), "DAG table check failed"


def _emit_min(eng, out, a, sa, b, sb, axis, L=56):
    """out = min(roll(a, sa, axis), roll(b, sb, axis)) on [128, cc, H, W]
    tiles; roll(t, s)[i] = t[(i+s) mod L]. Circular wrap via region split."""
    sa %= L
    sb %= L
    cuts = sorted({0, (L - sa) % L, (L - sb) % L})
    for idx, p in enumerate(cuts):
        q = cuts[idx + 1] if idx + 1 < len(cuts) else L
        n = q - p
        alo = (p + sa) % L
        blo = (p + sb) % L
        if axis == "w":
            eng.tensor_tensor(
                out=out[:, :, :, p:q],
                in0=a[:, :, :, alo:alo + n],
                in1=b[:, :, :, blo:blo + n],
                op=MIN,
            )
        else:
            of = out.rearrange("p c h w -> p c (h w)")
            af = a.rearrange("p c h w -> p c (h w)")
            bf = b.rearrange("p c h w -> p c (h w)")
            eng.tensor_tensor(
                out=of[:, :, p * W:q * W],
                in0=af[:, :, alo * W:(alo + n) * W],
                in1=bf[:, :, blo * W:(blo + n) * W],
                op=MIN,
            )


def _build(b_loc=B_LOC, cc=CC, oc=OC, act=GELU):
    nc = bacc.Bacc(None, target_bir_lowering=False)
    kc = 2 * cc

    xd = nc.dram_tensor("x_in", [b_loc, 128, cc, HW], BF16, kind="ExternalInput")
    x1d = nc.dram_tensor("x1_in", [b_loc, 128, cc, HW], BF16, kind="ExternalInput")
    wd = nc.dram_tensor("w_in", [kc, 128, oc * 128], BF16, kind="ExternalInput")
    idd = nc.dram_tensor("i_in", [1, 128, 128], BF16, kind="ExternalInput")
    pd = nc.dram_tensor("p_in", [oc, 128, 2], F32, kind="ExternalInput")
    yd = nc.dram_tensor("y_out", [b_loc, oc, 128, HW], BF16, kind="ExternalOutput")

    # liveness-based slot assignment for chain temporaries
    last_use = {}
    for i, (out, a, sa, b_, sb, axis) in enumerate(CHAIN_OPS):
        last_use[a] = i
        last_use[b_] = i
    last_use["h4"] = last_use["w4"] = len(CHAIN_OPS)  # used by merge pieces
    slot_of = {}
    free = []
    nslots = 0
    for i, (out, a, sa, b_, sb, axis) in enumerate(CHAIN_OPS):
        if free:
            slot_of[out] = free.pop()
        else:
            slot_of[out] = nslots
            nslots += 1
        for src in {a, b_}:
            if (src in slot_of and last_use.get(src) == i
                    and slot_of[src] not in free):
                free.append(slot_of[src])

    with TileContext(nc) as tc:
        with tc.tile_pool(name="sb", bufs=1) as pool, \
             tc.tile_pool(name="ps", bufs=8, space="PSUM") as psum:

            # --- hoisted input loads for both samples ---
            # XP = a small duplicate of chunk 0, loaded first, so the DVE
            # starts on chunk-0 pieces of h0/h1 ~3us before full X lands.
            XP = pool.tile([128, 1, H, W], BF16, tag="xp", bufs=1, name="XP")
            nc.sync.dma_start(out=XP.rearrange("p c h w -> p c (h w)"),
                              in_=xd[0][:, 0:1, :])
            Xs, X1s = [], []
            for b in range(b_loc):
                X = pool.tile([128, cc, H, W], BF16, tag="X", bufs=2, name="X")
                Xf_ = X.rearrange("p c h w -> p c (h w)")
                if b == 0:
                    nc.sync.dma_start(out=Xf_[:, 1:2, :], in_=xd[b][:, 1:2, :])
                    nc.scalar.dma_start(out=Xf_[:, 2:3, :], in_=xd[b][:, 2:3, :])
                    nc.scalar.dma_start(out=Xf_[:, 0:1, :], in_=xd[b][:, 0:1, :])
                else:
                    nc.sync.dma_start(out=Xf_, in_=xd[b])
                X1 = pool.tile([128, cc, H, W], BF16, tag="X1", bufs=1, name="X1")
                X1f = X1.rearrange("p c h w -> p c (h w)")
                # X1 = roll(x, -1 in w) is prepared on the host as its own
                # aligned input: the 2-byte-shifted view of x_in DMA'd at
                # ~half bandwidth (6270B misaligned packets) and needed a
                # col-55 fixup op; the aligned load needs neither.
                nc.sync.dma_start(out=X1f, in_=x1d[b])
                Xs.append(X)
                X1s.append(X1)

            # --- weights + params on the scalar HWDGE queue ---
            w_sb = pool.tile([128, kc, oc * 128], BF16, tag="w", bufs=1, name="w_sb")
            for k in range(kc):
                nc.scalar.dma_start(out=w_sb[:, k, :], in_=wd[k])
            prm = pool.tile([128, oc, 2], F32, tag="prm", bufs=1, name="prm")
            nc.scalar.dma_start(out=prm, in_=pd.rearrange("o p t -> p o t"))
            id_sb = pool.tile([128, 128], BF16, tag="id", bufs=1, name="id_sb")
            nc.scalar.dma_start(out=id_sb, in_=idd[0])

            y_sbs = [pool.tile([128, HW], BF16, tag="y", bufs=3,
                               name=f"y{o}") for o in range(oc)]
            yx = pool.tile([128, oc, HW], BF16, tag="yx", bufs=1, name="yx")
            Xfs = [Xs[b].rearrange("p c h w -> p c (h w)")
                   for b in range(b_loc)]
            chain_tiles = [None] * b_loc
            mp_slabs = [[None] * NT for _ in range(b_loc)]

            def emit_chain_op(b, op):
                out, a, sa, b_, sb, axis = op
                tiles = chain_tiles[b]
                t = pool.tile([128, cc, H, W], BF16,
                              tag=f"s{slot_of[out]}", bufs=1,
                              name=f"s{slot_of[out]}")
                if b == 0 and out in ("h0", "h1"):
                    # chunk-0 piece from the early XP copy; rest from X
                    _emit_min(nc.vector, t[:, 0:1], XP, sa, XP, sb, axis)
                    _emit_min(nc.vector, t[:, 1:3], Xs[b][:, 1:3], sa,
                              Xs[b][:, 1:3], sb, axis)
                else:
                    _emit_min(nc.vector, t, tiles[a], sa, tiles[b_], sb,
                              axis)
                tiles[out] = t

            def emit_slab(b, n):
                r0, r1 = n * HS, (n + 1) * HS
                w4t, h4t = chain_tiles[b]["w4"], chain_tiles[b]["h4"]
                mmt = pool.tile([128, cc, HS, W], BF16, tag="mmt",
                                bufs=2, name="mmt")
                nc.vector.tensor_tensor(
                    out=mmt, in0=w4t[:, :, r0:r1], in1=h4t[:, :, r0:r1],
                    op=MIN)
                mpt = pool.tile([128, cc, HS, W], BF16, tag=f"mp{n}",
                                bufs=1, name=f"mp{n}")
                nc.vector.tensor_tensor(
                    out=mpt, in0=mmt, in1=Xs[b][:, :, r0:r1], op=MIN)
                mp_slabs[b][n] = mpt

            def emit_xwave(b, n):
                # x-part partial sums for wave n -> PSUM -> ACT spill into
                # the yx region for wave n (bf16)
                for o in range(oc):
                    pst = psum.tile([128, NF], F32, tag="ps",
                                    name=f"x{b}_{n}_{o}")
                    for k in range(cc):
                        nc.tensor.matmul(
                            pst,
                            lhsT=w_sb[:, k, o * 128:(o + 1) * 128],
                            rhs=Xfs[b][:, k, n * NF:(n + 1) * NF],
                            start=(k == 0),
                            stop=(k == cc - 1),
                        )
                    nc.scalar.copy(
                        out=yx[:, o, n * NF:(n + 1) * NF], in_=pst)

            def emit_mwave(b, n):
                # identity-accumulate of the spilled x-part first (frees the
                # yx region for the next sample), then the 3 m-chunk matmuls
                # gated by the merge slab, then gelu + output DMA.
                mslab = mp_slabs[b][n].rearrange("p c h w -> p c (h w)")
                sl = slice(n * NF, (n + 1) * NF)
                for o in range(oc):
                    pst = psum.tile([128, NF], F32, tag="ps",
                                    name=f"u{b}_{n}_{o}")
                    nc.tensor.matmul(
                        pst, lhsT=id_sb, rhs=yx[:, o, sl],
                        start=True, stop=False,
                    )
                    for k in range(cc, kc):
                        nc.tensor.matmul(
                            pst,
                            lhsT=w_sb[:, k, o * 128:(o + 1) * 128],
                            rhs=mslab[:, k - cc],
                            start=False,
                            stop=(k == kc - 1),
                        )
                    nc.scalar.activation(
                        out=y_sbs[o][:, sl],
                        in_=pst,
                        func=act,
                        bias=prm[:, o, 1:2],
                        scale=prm[:, o, 0:1],
                    )
                    nc.sync.dma_start(out=yd[b, o][:, sl],
                                      in_=y_sbs[o][:, sl])

            # --- interleaved schedule ---
            # s0 chain + s0 x-phase; then s0's merge slabs are spread across
            # s1's chain emission so the PE gets slab-gated work throughout
            # s1's chain window (stays HAM-warm into s1's m-phase); s1's
            # x-waves interleave with s0's m-waves (per-wave yx region
            # handoff: s1's spill for wave n waits only on s0's identity
            # matmul for wave n).
            chain_tiles[0] = {"X": Xs[0], "X1": X1s[0]}
            for op in CHAIN_OPS:
                emit_chain_op(0, op)
            for n in range(NT):
                emit_xwave(0, n)
            chain_tiles[1] = {"X": Xs[1], "X1": X1s[1]}
            slab_sched = {3: [0], 4: [1], 5: [2], 6: [3], 7: [4], 8: [5],
                          9: [6]}
            for i, op in enumerate(CHAIN_OPS):
                emit_chain_op(1, op)
                for n in slab_sched.get(i, []):
                    emit_slab(0, n)
            for n in range(NT):
                emit_mwave(0, n)
                emit_xwave(1, n)
            for n in range(NT):
                emit_slab(1, n)
            for n in range(NT):
                emit_mwave(1, n)
    nc.finalize()
    return nc


_CACHE = {}


def _get_program():
    if "nc" not in _CACHE:
        _dag_check()
        _CACHE["nc"] = _build()
    return _CACHE["nc"]


def kernel(x, conv_w, conv_b, bn_scale, bn_bias, bn_mean, bn_var,
           _trace=False, _tmpdir=None):
    global LAST_RESULTS
    x = np.asarray(x, dtype=np.float32)
    conv_w = np.asarray(conv_w, dtype=np.float32)
    conv_b = np.asarray(conv_b, dtype=np.float32)
    bn_scale = np.asarray(bn_scale, dtype=np.float32)
    bn_bias = np.asarray(bn_bias, dtype=np.float32)
    bn_mean = np.asarray(bn_mean, dtype=np.float32)
    bn_var = np.asarray(bn_var, dtype=np.float32)

    # host-side weight/param folding
    Wm = conv_w[:, :, 0, 0]                      # [384, 768]
    W1, W2 = Wm[:, :C], Wm[:, C:]
    wT = np.concatenate([(W1 + W2).T, (-W2).T], axis=0)   # [768, 384]
    wd_arr = np.ascontiguousarray(
        wT.reshape(2 * CC, 128, OC * 128).astype(ml_dtypes.bfloat16)
    )
    inv = 1.0 / np.sqrt(bn_var + 1e-5)
    a = (inv * bn_scale).astype(np.float32)
    b_aff = ((conv_b - bn_mean) * a + bn_bias).astype(np.float32)
    prm_arr = np.ascontiguousarray(
        np.stack([a.reshape(OC, 128), b_aff.reshape(OC, 128)], axis=-1)
    )
    id_arr = np.ascontiguousarray(
        np.eye(128, dtype=np.float32).reshape(1, 128, 128)
        .astype(ml_dtypes.bfloat16)
    )

    # [B, CC, 128, HW] -> [B, 128, CC, HW] so the device DMA is contiguous
    xs = np.ascontiguousarray(
        x.reshape(B, CC, 128, HW).transpose(0, 2, 1, 3)
        .astype(ml_dtypes.bfloat16)
    )
    # x1 = roll(x, -1 along w), wrap included, as an aligned input tensor
    x1s = np.ascontiguousarray(
        np.roll(xs.reshape(B, 128, CC, H, W), -1, axis=4).reshape(B, 128, CC, HW)
    )
    in_maps = []
    for core in range(N_CORES):
        sl = slice(core * B_LOC, (core + 1) * B_LOC)
        in_maps.append({"x_in": np.ascontiguousarray(xs[sl]),
                        "x1_in": np.ascontiguousarray(x1s[sl]),
                        "w_in": wd_arr, "p_in": prm_arr, "i_in": id_arr})

    nc = _get_program()
    res = run_bass_kernel_spmd(
        nc, in_maps, core_ids=list(range(N_CORES)), trace=_trace, tmpdir=_tmpdir
    )
    LAST_RESULTS = res
    y = np.concatenate([r["y_out"] for r in res.results], axis=0)
    return y.astype(np.float32).reshape(B, C, H, W)

